# revision 1
# baseline (speedup 1.0000x reference)
"""Trainium2 Bass kernel for nn_BioSimulator.

Math: out[b,h,w] = clip(2 * sum_n Bw[b,n] * exp(-((px-vx[n])^2+(py-vy[n])^2)
                        * deg2pix^2 / (2*sigma_px[b,n]^2)), 0, 1)

px varies only along w and py only along h, so the Gaussian separates:
    exp(-(dx^2+dy^2)*c) = exp(-dx^2*c) * exp(-dy^2*c)
and the sum over points becomes a matmul over the point axis:
    out[b].T = Gx^T @ (2*Bw*Gy)        (transposed-output formulation)

Sharding: batch (2) x point-shards (4): each of the 8 cores handles one batch
and 256 of the N=1024 points (two 128-point partition tiles, accumulated in
PSUM across the two tiles).  Each core emits an unclipped partial
[2(wc),128(wp),256(h)]; the host sums the 4 shards per batch, transposes, and
clips.

Device per core:
  - DMA in pp[128,4] (stimulation + sigma scale, one column per point-tile)
    and sqd0/sqd1[128,512] = -0.5*[((xs-vx)*d2p)^2 | ((ys-vy)*d2p)^2].
  - Neuron math on [128,2] tiles (sigmoid via 1/(1+exp(-x)) so only the
    exp_and_others ACT table set is ever loaded; no sqrt needed because
    max(sqrt(v),1)^2 == max(v,1) for v>=0).
  - Per point-tile: one fused Exp [128,512] -> Gx|Gy in fp32r (rounded fp32:
    full-rate matmuls when the moving dim is >=256, near-fp32 accuracy,
    fp32 exponent range), scale Gy by 2*Bw, two PSUM-accumulating matmuls
    (w-chunks), copy out via DVE/ACT in parallel, DMA on both HWDGE rings.
"""

import numpy as np

import concourse.bass as bass
import concourse.bacc as bacc
import concourse.mybir as mybir
from concourse import tile
from concourse.bass_utils import run_bass_kernel_spmd

N_CORES = 8
NSHARDS = 4        # point shards per batch
PPC = 256          # points per core
NPT = 128          # points per partition tile
B = 2
H = W = 256

SPREAD = 0.000675
R2S = 0.5
SLOPE = 19152642.5
HALF = 1.057e-07
RHEO = 2.39e-05
FREQ = 300.0
PW = 0.00017
I_SCALE = 8e-05

F32 = mybir.dt.float32
F16 = mybir.dt.float16
F32R = mybir.dt.float32r
ALU = mybir.AluOpType
ACT = mybir.ActivationFunctionType

_NC = None


def _build_nc():
    nc = bacc.Bacc(None, target_bir_lowering=False, debug=False,
                   num_devices=N_CORES)
    pp = nc.dram_tensor("pp", [NPT, 4], F32, kind="ExternalInput")
    sqd0 = nc.dram_tensor("sqd0", [NPT, 2 * W], F32, kind="ExternalInput")
    sqd1 = nc.dram_tensor("sqd1", [NPT, 2 * W], F32, kind="ExternalInput")
    partial = nc.dram_tensor("partial", [2, 128, W], F32, kind="ExternalOutput")

    with tile.TileContext(nc) as tc:
        with (
            tc.tile_pool(name="const", bufs=1) as cpool,
            tc.tile_pool(name="work", bufs=2) as wpool,
            tc.tile_pool(name="obuf", bufs=2) as opool,
            tc.tile_pool(name="psum", bufs=2, space="PSUM") as psum,
        ):
            ppt = cpool.tile([NPT, 4], F32)
            nc.sync.dma_start(ppt[:], pp[:])
            sqdt = [cpool.tile([NPT, 2 * W], F32, tag=f"sqd{p}", name=f"sqdt{p}") for p in range(2)]
            nc.sync.dma_start(sqdt[0][:], sqd0[:])
            nc.sync.dma_start(sqdt[1][:], sqd1[:])

            # Cold-start absorber: a throwaway matmul on data that is ready
            # long before the real ones (PE is idle until ~3.7us otherwise),
            # so the real matmuls run at the warm clock with no LDW stall.
            wdum = cpool.tile([NPT, 2], F32)
            nc.vector.memset(wdum[:], 0.0)
            psd = psum.tile([2, 64], F32, tag="psd", name="psd", bufs=1)
            nc.tensor.matmul(psd[:], wdum[:], sqdt[0][:, 0:64], start=True, stop=True)
            # Table-load anchor: the exp table set loads before the first
            # ACTIVATE; give it one with no input-DMA dependency so the
            # ~1.3us load overlaps the input DMA instead of following it.
            dume = cpool.tile([NPT, 2], F32)
            nc.scalar.activation(dume[:], wdum[:], ACT.Exp)

            # -- Bw = sigmoid(SLOPE*(Q-HALF)).  The relu inside Q is replaced
            # exactly by clamping Bw from below: 1/(1+exp(A(s-t0)+C)) is
            # increasing in s and equals BW0 = 1/(1+e^C) at the threshold, so
            # Bw = max(1/(1+exp(A*s + (C-A*t0))), BW0).  The affine rides the
            # activation (bias memset at t=0), so the e-exp waits only on the
            # input DMA -- no DVE op ahead of it.
            bbias = cpool.tile([NPT, 1], F32)
            nc.vector.memset(bbias[:], float(SLOPE * (HALF + PW * FREQ * RHEO)))
            e = cpool.tile([NPT, 2], F32)
            nc.scalar.activation(
                e[:], ppt[:, 0:2], ACT.Exp,
                bias=bbias[:], scale=float(-SLOPE * PW * FREQ * I_SCALE),
            )
            ope = cpool.tile([NPT, 2], F32)
            nc.vector.tensor_scalar(ope[:], e[:], 1.0, None, ALU.add)
            bwu = cpool.tile([NPT, 2], F32)
            nc.vector.reciprocal(bwu[:], ope[:])
            bw = cpool.tile([NPT, 2], F32)
            nc.vector.tensor_scalar(
                bw[:], bwu[:], float(1.0 / (1.0 + np.exp(SLOPE * HALF))), None,
                ALU.max,
            )

            # -- negc = 1/max(sigma_px^2, 1); sigma_px^2 = stim*minv2sc comes
            # pre-scaled from the host (constant per-point factor), and the
            # -0.5 is baked into sqd, so exp(sqd * negc) is the Gaussian.
            v = cpool.tile([NPT, 2], F32)
            nc.vector.tensor_scalar(v[:], ppt[:, 2:4], 1.0, None, ALU.max)
            negc = cpool.tile([NPT, 2], F32)
            nc.vector.reciprocal(negc[:], v[:])

            # Per point-tile Gaussians; PSUM accumulates over the two tiles.
            pss = [psum.tile([128, W], F32, tag=f"ps{wc}", name=f"ps{wc}") for wc in range(2)]
            for p in range(2):
                gxy = wpool.tile([NPT, 2 * W], F32R, tag="gxy")
                nc.scalar.activation(
                    gxy[:], sqdt[p][:], ACT.Exp, scale=negc[:, p:p + 1],
                )
                gys = wpool.tile([NPT, W], F32R, tag="gys")
                nc.vector.tensor_scalar(
                    gys[:], gxy[:, W:2 * W], bw[:, p:p + 1], 2.0, ALU.mult, ALU.mult
                )
                # Transposed formulation: stationary = Gx chunk (ready before
                # gys), moving = gys; LDWEIGHTS stays off the critical path.
                for wc in range(2):
                    nc.tensor.matmul(
                        pss[wc][:],
                        gxy[:, wc * 128:(wc + 1) * 128],
                        gys[:],
                        start=(p == 0), stop=(p == 1),
                    )
            for wc in range(2):
                ob = opool.tile([128, W], F32)
                # Copies split across DVE and ACT so they run concurrently;
                # each DMA goes out on its issuer's HWDGE ring.
                if wc == 0:
                    nc.vector.tensor_copy(ob[:], pss[wc][:])
                    nc.sync.dma_start(partial[wc], ob[:])
                else:
                    nc.scalar.copy(ob[:], pss[wc][:])
                    nc.scalar.dma_start(partial[wc], ob[:])
    nc.compile()
    return nc


def _get_nc():
    global _NC
    if _NC is None:
        _NC = _build_nc()
    return _NC


def make_in_maps(stimulation, vx, vy, M, px, py, idx):
    stimulation = np.asarray(stimulation, dtype=np.float32)
    vx = np.asarray(vx, dtype=np.float32)
    vy = np.asarray(vy, dtype=np.float32)
    M = np.asarray(M, dtype=np.float32)
    px = np.asarray(px, dtype=np.float32)
    py = np.asarray(py, dtype=np.float32)
    idx = np.asarray(idx)

    fov = np.float32(px.max())
    deg2pix = np.float32(W) / (fov * np.float32(2.0))
    xs = px[0, :]            # px[h,w] = xs[w]
    ys = py[:, 0]            # py[h,w] = ys[h]
    flat = stimulation.reshape(B, -1)[:, idx]          # [B, N]
    minv2sc = (I_SCALE / SPREAD) * (R2S * deg2pix / M) ** 2  # [N]

    def sqd_for(sl):
        dx = (xs[None, :] - vx[sl, None]) * deg2pix    # [NPT, W]
        dy = (ys[None, :] - vy[sl, None]) * deg2pix    # [NPT, H]
        # -0.5 baked in: exponent = sqd * (1/max(sigma_px^2, 1))
        out = np.concatenate([dx * dx, dy * dy], axis=1) * np.float32(-0.5)
        return np.ascontiguousarray(out, dtype=np.float32)

    in_maps = []
    for c in range(N_CORES):
        b, s = divmod(c, NSHARDS)
        sl0 = slice(s * PPC, s * PPC + NPT)
        sl1 = slice(s * PPC + NPT, (s + 1) * PPC)
        pp = np.zeros((NPT, 4), np.float32)
        pp[:, 0] = flat[b, sl0]
        pp[:, 1] = flat[b, sl1]
        pp[:, 2] = flat[b, sl0] * minv2sc[sl0]
        pp[:, 3] = flat[b, sl1] * minv2sc[sl1]
        in_maps.append({
            "pp": pp,
            "sqd0": sqd_for(sl0),
            "sqd1": sqd_for(sl1),
        })
    return in_maps


def combine(results):
    acc = np.zeros((B, H, W), np.float32)
    for c, r in enumerate(results):
        b = c // NSHARDS
        # device emits out'[wc, wp, h]; out[b, h, wc*128+wp] = out'[...]
        p = r["partial"]
        acc[b] += p.transpose(2, 0, 1).reshape(H, W)
    return np.clip(acc, 0.0, 1.0)[:, None, :, :].astype(np.float32)


def kernel(stimulation, vx, vy, M, px, py, idx):
    nc = _get_nc()
    in_maps = make_in_maps(stimulation, vx, vy, M, px, py, idx)
    res = run_bass_kernel_spmd(nc, in_maps, list(range(N_CORES)))
    return combine(res.results)



# revision 7
# speedup vs baseline: 1.2350x; 1.2350x over previous
"""Trainium2 Bass kernel for nn_BioSimulator (raw-Bass, manual semaphores).

Math: out[b,h,w] = clip(2 * sum_n Bw[b,n] * exp(-((px-vx[n])^2+(py-vy[n])^2)
                        * deg2pix^2 / (2*sigma_px[b,n]^2)), 0, 1)

px varies only along w and py only along h, so the Gaussian separates:
    exp(-(dx^2+dy^2)*c) = exp(-dx^2*c) * exp(-dy^2*c)
and the sum over points becomes a matmul over the point axis:
    out[b].T = Gx^T @ (2*Bw*Gy)        (transposed-output formulation)

Sharding: batch (2) x point-shards (4): each of the 8 cores handles one batch
and 256 of the N=1024 points (two 128-point partition tiles, accumulated in
PSUM across the two tiles).  Each core emits an unclipped partial
[2(wc),128(wp),256(h)]; the host sums the 4 shards per batch, transposes, and
clips.

Host prep folds every per-point scalar into the exponent tables it already
builds (the baseline precomputed -0.5*d2p^2*(xs-vx)^2 tables and pre-scaled
sigma; this folds negc = 1/max(sigma_px^2,1) and ln(2*Bw) in as well), so the
device program is exactly:
    sq0,sq1 --DMA--> Exp([128,512]) x2 --> 4 PSUM-accumulating matmuls
    --> 2 PSUM->SBUF copies --> 2 output DMAs
with every engine-to-engine edge synchronized by hand-placed semaphores.

Raw Bass (no TileContext) is used because the tile framework's epilogue
(sem RANGE_CLEAR + two all-engine barriers) costs ~700ns after the last
output DMA; with manual sems the program ends when the output DMA lands.

DMA placement: the two halves of sq0 go on the SP and ACT queues in parallel
(each 500ns min-cost) so the tile-0 exponent table is in SBUF at the earliest
possible 2417ns; sq1 rides the DVE queue (790ns) and lands at 2707ns, before
the second Exp needs it at ~3129ns.  The output DMAs are issued from the same
engines that do the PSUM->SBUF eviction (DVE for chunk 0, ACT for chunk 1) so
no cross-engine semaphore hop sits between eviction and DMA.
"""

import numpy as np

import concourse.bass as bass
import concourse.bacc as bacc
import concourse.mybir as mybir
from concourse.bass_utils import run_bass_kernel_spmd

N_CORES = 8
NSHARDS = 4        # point shards per batch
PPC = 256          # points per core
NPT = 128          # points per partition tile
B = 2
H = W = 256

SPREAD = 0.000675
R2S = 0.5
SLOPE = 19152642.5
HALF = 1.057e-07
RHEO = 2.39e-05
FREQ = 300.0
PW = 0.00017
I_SCALE = 8e-05

F32 = mybir.dt.float32
F32R = mybir.dt.float32r
ACT = mybir.ActivationFunctionType

_NC = None


def _build_nc():
    nc = bacc.Bacc(None, target_bir_lowering=False, debug=False,
                   num_devices=N_CORES)
    sq0 = nc.dram_tensor("sq0", [NPT, 2 * W], F32, kind="ExternalInput")
    sq1 = nc.dram_tensor("sq1", [NPT, 2 * W], F32, kind="ExternalInput")
    partial = nc.dram_tensor("partial", [2, 128, W], F32, kind="ExternalOutput")

    sq0t = nc.alloc_sbuf_tensor("sq0t", [NPT, 2 * W], F32)
    sq1t = nc.alloc_sbuf_tensor("sq1t", [NPT, 2 * W], F32)
    gxy0 = nc.alloc_sbuf_tensor("gxy0", [NPT, 2 * W], F32R)
    gxy1 = nc.alloc_sbuf_tensor("gxy1", [NPT, 2 * W], F32R)
    ob0 = nc.alloc_sbuf_tensor("ob0", [128, W], F32)
    ob1 = nc.alloc_sbuf_tensor("ob1", [128, W], F32)
    dume = nc.alloc_sbuf_tensor("dume", [128, 1], F32)
    ps0 = nc.alloc_psum_tensor("ps0", [128, W], F32)
    ps1 = nc.alloc_psum_tensor("ps1", [128, W], F32)

    si0a = nc.alloc_semaphore("si0a")
    si0b = nc.alloc_semaphore("si0b")
    si1 = nc.alloc_semaphore("si1")
    se0 = nc.alloc_semaphore("se0")
    se1 = nc.alloc_semaphore("se1")
    sp0 = nc.alloc_semaphore("sp0")
    sp1 = nc.alloc_semaphore("sp1")
    sc0 = nc.alloc_semaphore("sc0")
    sc1 = nc.alloc_semaphore("sc1")
    so0 = nc.alloc_semaphore("so0")
    so1 = nc.alloc_semaphore("so1")

    # Input DMAs: sq0 split across the SP and ACT HWDGE queues (both land at
    # t~2417); sq1 whole on the DVE queue (lands ~2707, ahead of its ~3129
    # consumer).
    nc.sync.dma_start(sq0t[:, 0:W], sq0[:, 0:W]).then_inc(si0a, 16)
    nc.scalar.dma_start(sq0t[:, W:2 * W], sq0[:, W:2 * W]).then_inc(si0b, 16)
    nc.gpsimd.dma_start(sq1t[:], sq1[:]).then_inc(si1, 16)

    # Table-load anchor: first ACT activation pulls in the exp table set
    # (1283ns); anchoring it on a const input makes the load overlap the
    # input DMAs instead of following them.
    zero = nc.const_aps.aps[(mybir.dt.float32, 0.0)]
    nc.scalar.activation(dume.ap(), zero, ACT.Exp)

    # Exponent tables arrive fully folded: Gx|2Bw*Gy = exp(sq) directly.
    nc.scalar.wait_ge(si0a, 16)
    nc.scalar.wait_ge(si0b, 16)
    nc.scalar.activation(gxy0[:], sq0t[:], ACT.Exp).then_inc(se0, 1)
    nc.scalar.wait_ge(si1, 16)
    nc.scalar.activation(gxy1[:], sq1t[:], ACT.Exp).then_inc(se1, 1)

    # out'[w, h] = sum_p Gx[p, w] * (2Bw*Gy)[p, h], accumulated over the two
    # point tiles in PSUM; two 128-wide w chunks (PSUM partition limit).
    nc.tensor.wait_ge(se0, 1)
    nc.tensor.matmul(ps0.ap(), gxy0[:, 0:128], gxy0[:, W:2 * W],
                     start=True, stop=False)
    nc.tensor.matmul(ps1.ap(), gxy0[:, 128:W], gxy0[:, W:2 * W],
                     start=True, stop=False)
    nc.tensor.wait_ge(se1, 1)
    nc.tensor.matmul(ps0.ap(), gxy1[:, 0:128], gxy1[:, W:2 * W],
                     start=False, stop=True).then_inc(sp0, 1)
    nc.tensor.matmul(ps1.ap(), gxy1[:, 128:W], gxy1[:, W:2 * W],
                     start=False, stop=True).then_inc(sp1, 1)

    # Evict+store: chunk 0 evicts on DVE (DMA issued from SP, the only other
    # free HWDGE queue); chunk 1 evicts on ACT which issues its own DMA in
    # program order.
    nc.vector.wait_ge(sp0, 1)
    nc.vector.tensor_copy(ob0.ap(), ps0.ap()).then_inc(sc0, 1)
    nc.sync.wait_ge(sc0, 1)
    nc.sync.dma_start(partial[0], ob0.ap()).then_inc(so0, 16)
    nc.scalar.wait_ge(sp1, 1)
    nc.scalar.copy(ob1.ap(), ps1.ap()).then_inc(sc1, 1)
    nc.scalar.wait_ge(sc1, 1)
    nc.scalar.dma_start(partial[1], ob1.ap()).then_inc(so1, 16)

    # Keep the program alive until the output DMAs land.
    nc.sync.wait_ge(so0, 16)
    nc.sync.wait_ge(so1, 16)

    nc.compile()
    return nc


def _get_nc():
    global _NC
    if _NC is None:
        _NC = _build_nc()
    return _NC


def make_in_maps(stimulation, vx, vy, M, px, py, idx):
    stimulation = np.asarray(stimulation, dtype=np.float32)
    vx = np.asarray(vx, dtype=np.float64)
    vy = np.asarray(vy, dtype=np.float64)
    M = np.asarray(M, dtype=np.float64)
    px = np.asarray(px, dtype=np.float32)
    py = np.asarray(py, dtype=np.float32)
    idx = np.asarray(idx)

    fov = np.float64(px.max())
    deg2pix = np.float64(W) / (fov * 2.0)
    xs = px[0, :].astype(np.float64)     # px[h,w] = xs[w]
    ys = py[:, 0].astype(np.float64)     # py[h,w] = ys[h]
    flat = stimulation.reshape(B, -1)[:, idx].astype(np.float64)  # [B, N]

    I = flat * I_SCALE                                    # [B, N]
    sig_px2 = (I / SPREAD) * (R2S * deg2pix / M[None, :]) ** 2
    negc = -0.5 / np.maximum(sig_px2, 1.0)                # [B, N]
    Q = np.maximum(I - RHEO, 0.0) * PW * FREQ
    Bw = 1.0 / (1.0 + np.exp(-SLOPE * (Q - HALF)))        # [B, N]
    ln2bw = np.log(2.0 * Bw)                              # [B, N]

    in_maps = []
    for c in range(N_CORES):
        b, s = divmod(c, NSHARDS)

        def sq_for(sl):
            dx2 = ((xs[None, :] - vx[sl, None]) * deg2pix) ** 2   # [NPT, W]
            dy2 = ((ys[None, :] - vy[sl, None]) * deg2pix) ** 2   # [NPT, H]
            cc = negc[b, sl][:, None]
            out = np.concatenate(
                [dx2 * cc, dy2 * cc + ln2bw[b, sl][:, None]], axis=1)
            return np.ascontiguousarray(out, dtype=np.float32)

        sl0 = slice(s * PPC, s * PPC + NPT)
        sl1 = slice(s * PPC + NPT, (s + 1) * PPC)
        in_maps.append({"sq0": sq_for(sl0), "sq1": sq_for(sl1)})
    return in_maps


def combine(results):
    acc = np.zeros((B, H, W), np.float32)
    for c, r in enumerate(results):
        b = c // NSHARDS
        # device emits out'[wc, wp, h]; out[b, h, wc*128+wp] = out'[...]
        p = r["partial"]
        acc[b] += p.transpose(2, 0, 1).reshape(H, W)
    return np.clip(acc, 0.0, 1.0)[:, None, :, :].astype(np.float32)


def kernel(stimulation, vx, vy, M, px, py, idx):
    nc = _get_nc()
    in_maps = make_in_maps(stimulation, vx, vy, M, px, py, idx)
    res = run_bass_kernel_spmd(nc, in_maps, list(range(N_CORES)))
    return combine(res.results)


# revision 10
# speedup vs baseline: 1.3169x; 1.0663x over previous
"""Trainium2 Bass kernel for nn_BioSimulator (raw-Bass, manual semaphores).

Math: out[b,h,w] = clip(2 * sum_n Bw[b,n] * exp(-((px-vx[n])^2+(py-vy[n])^2)
                        * deg2pix^2 / (2*sigma_px[b,n]^2)), 0, 1)

px varies only along w and py only along h, so the Gaussian separates:
    exp(-(dx^2+dy^2)*c) = exp(-dx^2*c) * exp(-dy^2*c)
and the sum over points becomes a matmul over the point axis:
    out[b].T = Gx^T @ (2*Bw*Gy)        (transposed-output formulation)

Sharding: batch (2) x point-shards (4): each of the 8 cores handles one batch
and 256 of the N=1024 points (two 128-point partition tiles, accumulated in
PSUM across the two tiles).  Each core emits an unclipped partial
[2(wc),128(wp),256(h)]; the host sums the 4 shards per batch, transposes, and
clips.

Host prep folds every per-point scalar into the exponent tables it already
builds (the baseline precomputed -0.5*d2p^2*(xs-vx)^2 tables and pre-scaled
sigma; this folds negc = 1/max(sigma_px^2,1) and ln(2*Bw) in as well), so the
device program is exactly:
    sq0,sq1 --DMA--> Exp([128,512]) x2 --> 4 PSUM-accumulating matmuls
    --> 2 PSUM->SBUF copies --> 2 output DMAs
with every engine-to-engine edge synchronized by hand-placed semaphores.

Raw Bass (no TileContext) is used because the tile framework's epilogue
(sem RANGE_CLEAR + two all-engine barriers) costs ~700ns after the last
output DMA; with manual sems the program ends when the output DMA lands.

DMA placement: the two halves of sq0 go on the SP and ACT queues in parallel
(each 500ns min-cost) so the tile-0 exponent table is in SBUF at the earliest
possible 2417ns; sq1 rides the DVE queue (790ns) and lands at 2707ns, before
the second Exp needs it at ~3129ns.  The output DMAs are issued from the same
engines that do the PSUM->SBUF eviction (DVE for chunk 0, ACT for chunk 1) so
no cross-engine semaphore hop sits between eviction and DMA.
"""

import numpy as np

import concourse.bass as bass
import concourse.bacc as bacc
import concourse.mybir as mybir
from concourse.bass_utils import run_bass_kernel_spmd

N_CORES = 8
NSHARDS = 4        # point shards per batch
PPC = 256          # points per core
NPT = 128          # points per partition tile
B = 2
H = W = 256

SPREAD = 0.000675
R2S = 0.5
SLOPE = 19152642.5
HALF = 1.057e-07
RHEO = 2.39e-05
FREQ = 300.0
PW = 0.00017
I_SCALE = 8e-05

F32 = mybir.dt.float32
F32R = mybir.dt.float32r
ACT = mybir.ActivationFunctionType

_NC = None


def _build_nc():
    nc = bacc.Bacc(None, target_bir_lowering=False, debug=False,
                   num_devices=N_CORES)
    sq0 = nc.dram_tensor("sq0", [NPT, 2 * W], F32, kind="ExternalInput")
    sq1 = nc.dram_tensor("sq1", [NPT, 2 * W], F32, kind="ExternalInput")
    partial = nc.dram_tensor("partial", [2, 128, W], F32, kind="ExternalOutput")

    sq0t = nc.alloc_sbuf_tensor("sq0t", [NPT, 2 * W], F32)
    sq1t = nc.alloc_sbuf_tensor("sq1t", [NPT, 2 * W], F32)
    gxy0 = nc.alloc_sbuf_tensor("gxy0", [NPT, 2 * W], F32R)
    gxy1 = nc.alloc_sbuf_tensor("gxy1", [NPT, 2 * W], F32R)
    ob0 = nc.alloc_sbuf_tensor("ob0", [128, W], F32)
    ob1 = nc.alloc_sbuf_tensor("ob1", [128, W], F32)
    dume = nc.alloc_sbuf_tensor("dume", [128, 1], F32)
    ps0 = nc.alloc_psum_tensor("ps0", [128, W], F32)
    ps1 = nc.alloc_psum_tensor("ps1", [128, W], F32)

    si0 = nc.alloc_semaphore("si0")
    si1 = nc.alloc_semaphore("si1")
    se0 = nc.alloc_semaphore("se0")
    se1 = nc.alloc_semaphore("se1")
    sp0 = nc.alloc_semaphore("sp0")
    sp1 = nc.alloc_semaphore("sp1")
    sc0 = nc.alloc_semaphore("sc0")
    sc1 = nc.alloc_semaphore("sc1")
    so0 = nc.alloc_semaphore("so0")
    so1 = nc.alloc_semaphore("so1")

    # Input DMAs stay off the ACT queue entirely (ACT's own queue time is the
    # critical path: table load -> Exp0 -> Exp1 -> eviction -> output DMA).
    nc.sync.dma_start(sq0t[:], sq0[:]).then_inc(si0, 16)
    nc.gpsimd.dma_start(sq1t[:], sq1[:]).then_inc(si1, 16)

    # Table-load anchor: first ACT activation pulls in the exp table set
    # (1283ns); anchoring it on a const input makes the load overlap the
    # input DMAs instead of following them.
    zero = nc.const_aps.aps[(mybir.dt.float32, 0.0)]
    nc.scalar.activation(dume.ap(), zero, ACT.Exp)

    # Exponent tables arrive fully folded: Gx|2Bw*Gy = exp(sq) directly.
    nc.scalar.wait_ge(si0, 16)
    nc.scalar.activation(gxy0[:], sq0t[:], ACT.Exp).then_inc(se0, 1)
    nc.scalar.wait_ge(si1, 16)
    nc.scalar.activation(gxy1[:], sq1t[:], ACT.Exp).then_inc(se1, 1)

    # out'[w, h] = sum_p Gx[p, w] * (2Bw*Gy)[p, h], accumulated over the two
    # point tiles in PSUM; two 128-wide w chunks (PSUM partition limit).
    nc.tensor.wait_ge(se0, 1)
    nc.tensor.matmul(ps0.ap(), gxy0[:, 0:128], gxy0[:, W:2 * W],
                     start=True, stop=False)
    nc.tensor.matmul(ps1.ap(), gxy0[:, 128:W], gxy0[:, W:2 * W],
                     start=True, stop=False)
    nc.tensor.wait_ge(se1, 1)
    nc.tensor.matmul(ps0.ap(), gxy1[:, 0:128], gxy1[:, W:2 * W],
                     start=False, stop=True).then_inc(sp0, 1)
    nc.tensor.matmul(ps1.ap(), gxy1[:, 128:W], gxy1[:, W:2 * W],
                     start=False, stop=True).then_inc(sp1, 1)

    # Evict+store: chunk 0 evicts on DVE (DMA issued from SP, the only other
    # free HWDGE queue); chunk 1 evicts on ACT which issues its own DMA in
    # program order.
    nc.vector.wait_ge(sp0, 1)
    nc.vector.tensor_copy(ob0.ap(), ps0.ap()).then_inc(sc0, 1)
    nc.sync.wait_ge(sc0, 1)
    nc.sync.dma_start(partial[0], ob0.ap()).then_inc(so0, 16)
    nc.scalar.wait_ge(sp1, 1)
    nc.scalar.copy(ob1.ap(), ps1.ap()).then_inc(sc1, 1)
    nc.scalar.wait_ge(sc1, 1)
    nc.scalar.dma_start(partial[1], ob1.ap()).then_inc(so1, 16)

    # Keep the program alive until the output DMAs land.
    nc.sync.wait_ge(so0, 16)
    nc.sync.wait_ge(so1, 16)

    nc.compile()
    return nc


def _get_nc():
    global _NC
    if _NC is None:
        _NC = _build_nc()
    return _NC


def make_in_maps(stimulation, vx, vy, M, px, py, idx):
    stimulation = np.asarray(stimulation, dtype=np.float32)
    vx = np.asarray(vx, dtype=np.float64)
    vy = np.asarray(vy, dtype=np.float64)
    M = np.asarray(M, dtype=np.float64)
    px = np.asarray(px, dtype=np.float32)
    py = np.asarray(py, dtype=np.float32)
    idx = np.asarray(idx)

    fov = np.float64(px.max())
    deg2pix = np.float64(W) / (fov * 2.0)
    xs = px[0, :].astype(np.float64)     # px[h,w] = xs[w]
    ys = py[:, 0].astype(np.float64)     # py[h,w] = ys[h]
    flat = stimulation.reshape(B, -1)[:, idx].astype(np.float64)  # [B, N]

    I = flat * I_SCALE                                    # [B, N]
    sig_px2 = (I / SPREAD) * (R2S * deg2pix / M[None, :]) ** 2
    negc = -0.5 / np.maximum(sig_px2, 1.0)                # [B, N]
    Q = np.maximum(I - RHEO, 0.0) * PW * FREQ
    Bw = 1.0 / (1.0 + np.exp(-SLOPE * (Q - HALF)))        # [B, N]
    ln2bw = np.log(2.0 * Bw)                              # [B, N]

    in_maps = []
    for c in range(N_CORES):
        b, s = divmod(c, NSHARDS)

        def sq_for(sl):
            dx2 = ((xs[None, :] - vx[sl, None]) * deg2pix) ** 2   # [NPT, W]
            dy2 = ((ys[None, :] - vy[sl, None]) * deg2pix) ** 2   # [NPT, H]
            cc = negc[b, sl][:, None]
            out = np.concatenate(
                [dx2 * cc, dy2 * cc + ln2bw[b, sl][:, None]], axis=1)
            return np.ascontiguousarray(out, dtype=np.float32)

        sl0 = slice(s * PPC, s * PPC + NPT)
        sl1 = slice(s * PPC + NPT, (s + 1) * PPC)
        in_maps.append({"sq0": sq_for(sl0), "sq1": sq_for(sl1)})
    return in_maps


def combine(results):
    acc = np.zeros((B, H, W), np.float32)
    for c, r in enumerate(results):
        b = c // NSHARDS
        # device emits out'[wc, wp, h]; out[b, h, wc*128+wp] = out'[...]
        p = r["partial"]
        acc[b] += p.transpose(2, 0, 1).reshape(H, W)
    return np.clip(acc, 0.0, 1.0)[:, None, :, :].astype(np.float32)


def kernel(stimulation, vx, vy, M, px, py, idx):
    nc = _get_nc()
    in_maps = make_in_maps(stimulation, vx, vy, M, px, py, idx)
    res = run_bass_kernel_spmd(nc, in_maps, list(range(N_CORES)))
    return combine(res.results)
